# revision 3
# baseline (speedup 1.0000x reference)
"""DiffAttn kernel for 8 trn2 NeuronCores.

Problem (per reference):
  X [4, 4096, 1024]; Wq/Wk [1024, 256]; Wv [1024, 128]; biases; lam scalar.
  Q,K = X@Wq+bq, X@Wk+bk ; V = X@Wv+bv
  A_i = Q_i @ K_i^T / sqrt(128)  (i = 1,2 : the two 128-wide halves)
  out = (softmax(A1) - lam * softmax(A2)) @ V          -> [4, 4096, 128]

Sharding: 8 cores = 4 batches x 2 query-halves. Each core computes the
attention output for 2048 queries of one batch; K/V projections for the
full 4096 keys of that batch are computed redundantly on both cores of the
pair (no collectives needed). Host passes X^T per core with the core's
query rows ordered first; key order is irrelevant to softmax as long as K
and V agree.

On-chip layout (per core):
  XT [1024, 4096]   (emb on partitions, streamed in 512-col chunks)
  QiT [128, 2048], KiT [128, 4096]  (head dim on partitions)
  V   [128, 32*128] (key idx on partitions per 128-chunk)
  scores computed transposed: S^T[sk, sq] = (K chunk)^T^T... = lhsT=KT chunk,
  rhs=QT chunk -> both natural layouts; exp on ScalarE; PV matmul with V as
  stationary, E^T as moving; softmax denominators accumulated on VectorE and
  normalization applied after the output transpose.
"""

import os
import sys

sys.path.insert(0, "/opt/trn_rl_repo")

import numpy as np

import concourse.bacc as bacc
import concourse.mybir as mybir
from concourse import masks
from concourse.tile import TileContext
from concourse.bass_utils import run_bass_kernel_spmd

F32 = mybir.dt.float32
# float32r: reduced-precision fp32 matmul mode, 4x the throughput of fp32 at
# free-dim >= 256. Set MM_F32R = False to fall back to exact fp32 matmuls.
MM_F32R = os.environ.get("KERNEL_MM_DT", "f32r") == "f32r"
MM_DT = mybir.dt.float32r if MM_F32R else F32

D = 128
EMB = 1024
B, S = 4, 4096
NQ = S // 2          # queries per core
SQC = 512            # query-column chunk (matmul moving dim)
NSQ = NQ // SQC      # 4 query chunks
NSK = S // 128       # 32 key tiles
NE = EMB // 128      # 8 emb tiles
NCC = S // SQC       # 8 projection column chunks
INV_SQRT_D = 1.0 / np.sqrt(np.float32(D))

# Trace/timing knob for the dev harness (test.py); graded path leaves it off.
TRACE = False
TRACE_DIR = None
LAST_RESULT = None


def _in(ap):
    """Bitcast a DRAM fp32 AP for DMA into an MM_DT tile."""
    return ap.bitcast(MM_DT) if MM_F32R else ap


def _build():
    nc = bacc.Bacc("TRN2", target_bir_lowering=False, debug=False, num_devices=8)

    xt = nc.dram_tensor("xt", [EMB, S], F32, kind="ExternalInput")
    wq = nc.dram_tensor("wq", [EMB, 2 * D], F32, kind="ExternalInput")
    wk = nc.dram_tensor("wk", [EMB, 2 * D], F32, kind="ExternalInput")
    wv = nc.dram_tensor("wv", [EMB, D], F32, kind="ExternalInput")
    bq = nc.dram_tensor("bq", [2 * D, 1], F32, kind="ExternalInput")
    bk = nc.dram_tensor("bk", [2 * D, 1], F32, kind="ExternalInput")
    bv = nc.dram_tensor("bv", [D, 1], F32, kind="ExternalInput")
    lamv = nc.dram_tensor("lamv", [128, 1], F32, kind="ExternalInput")
    out = nc.dram_tensor("o", [NQ, D], F32, kind="ExternalOutput")

    with TileContext(nc) as tc:
        from contextlib import ExitStack

        with ExitStack() as ctx:
            cpool = ctx.enter_context(tc.tile_pool(name="const", bufs=1))
            ident = cpool.tile([128, 128], F32)
            masks.make_identity(nc, ident[:])

            bq1 = cpool.tile([128, 1], F32, tag="bq1")
            bq2 = cpool.tile([128, 1], F32, tag="bq2")
            bk1 = cpool.tile([128, 1], F32, tag="bk1")
            bk2 = cpool.tile([128, 1], F32, tag="bk2")
            bvt = cpool.tile([128, 1], F32, tag="bvt")
            lam_t = cpool.tile([128, 1], F32, tag="lam")
            nc.sync.dma_start(out=bq1[:], in_=bq[0:128, :])
            nc.sync.dma_start(out=bq2[:], in_=bq[128:256, :])
            nc.sync.dma_start(out=bk1[:], in_=bk[0:128, :])
            nc.sync.dma_start(out=bk2[:], in_=bk[128:256, :])
            nc.sync.dma_start(out=bvt[:], in_=bv[0:128, :])
            nc.sync.dma_start(out=lam_t[:], in_=lamv[:, :])

            wpool = ctx.enter_context(tc.tile_pool(name="w", bufs=1))
            wq1 = wpool.tile([128, NE, 128], MM_DT, tag="wq1")
            wq2 = wpool.tile([128, NE, 128], MM_DT, tag="wq2")
            wk1 = wpool.tile([128, NE, 128], MM_DT, tag="wk1")
            wk2 = wpool.tile([128, NE, 128], MM_DT, tag="wk2")
            wvt = wpool.tile([128, NE, 128], MM_DT, tag="wvt")
            for t in range(NE):
                r = slice(t * 128, (t + 1) * 128)
                nc.sync.dma_start(out=wq1[:, t, :], in_=_in(wq[r, 0:128]))
                nc.sync.dma_start(out=wq2[:, t, :], in_=_in(wq[r, 128:256]))
                nc.sync.dma_start(out=wk1[:, t, :], in_=_in(wk[r, 0:128]))
                nc.sync.dma_start(out=wk2[:, t, :], in_=_in(wk[r, 128:256]))
                nc.sync.dma_start(out=wvt[:, t, :], in_=_in(wv[r, 0:128]))

            qkv = ctx.enter_context(tc.tile_pool(name="qkv", bufs=1))
            qt1 = qkv.tile([128, NQ], MM_DT, tag="qt1")
            qt2 = qkv.tile([128, NQ], MM_DT, tag="qt2")
            kt1 = qkv.tile([128, S], MM_DT, tag="kt1")
            kt2 = qkv.tile([128, S], MM_DT, tag="kt2")
            vv = qkv.tile([128, S], MM_DT, tag="vv")  # col c*128+j = V[key, d]

            # ---------------- projections ----------------
            with ExitStack() as pctx:
                xpool = pctx.enter_context(tc.tile_pool(name="xt", bufs=3))
                ppool = pctx.enter_context(
                    tc.tile_pool(name="ppsum", bufs=1, space="PSUM")
                )
                tpool = pctx.enter_context(
                    tc.tile_pool(name="ptr", bufs=2, space="PSUM")
                )
                vspool = pctx.enter_context(tc.tile_pool(name="vts", bufs=2))

                for cc in range(NCC):
                    csl = slice(cc * SQC, (cc + 1) * SQC)
                    xt_t = xpool.tile([128, NE, SQC], MM_DT, tag="xchunk")
                    for t in range(NE):
                        nc.sync.dma_start(
                            out=xt_t[:, t, :],
                            in_=_in(xt[t * 128 : (t + 1) * 128, csl]),
                        )

                    groups = [
                        (kt1, wk1, bk1, "k1"),
                        (kt2, wk2, bk2, "k2"),
                    ]
                    if cc < NSQ:
                        groups += [(qt1, wq1, bq1, "q1"), (qt2, wq2, bq2, "q2")]

                    for dst, w_t, b_t, tag in groups:
                        ps = ppool.tile([128, SQC], F32, tag=tag)
                        for t in range(NE):
                            nc.tensor.matmul(
                                ps[:],
                                w_t[:, t, :],
                                xt_t[:, t, :],
                                start=(t == 0),
                                stop=(t == NE - 1),
                            )
                        nc.vector.tensor_scalar_add(dst[:, csl], ps[:], b_t[:, 0:1])

                    # V^T chunk -> bias -> transpose into vv
                    ps = ppool.tile([128, SQC], F32, tag="vt")
                    for t in range(NE):
                        nc.tensor.matmul(
                            ps[:],
                            wvt[:, t, :],
                            xt_t[:, t, :],
                            start=(t == 0),
                            stop=(t == NE - 1),
                        )
                    vt_s = vspool.tile([128, SQC], F32, tag="vts")
                    nc.vector.tensor_scalar_add(vt_s[:], ps[:], bvt[:, 0:1])
                    for j in range(SQC // 128):
                        tr = tpool.tile([128, 128], F32, tag="vtr")
                        nc.tensor.transpose(
                            tr[:], vt_s[:, j * 128 : (j + 1) * 128], ident[:]
                        )
                        col = (cc * (SQC // 128) + j) * 128
                        nc.vector.tensor_copy(vv[:, col : col + 128], tr[:])

            # ---------------- attention ----------------
            with ExitStack() as actx:
                spsum = actx.enter_context(
                    tc.tile_pool(name="spsum", bufs=2, space="PSUM")
                )
                opsum = actx.enter_context(
                    tc.tile_pool(name="opsum", bufs=1, space="PSUM")
                )
                tpsum = actx.enter_context(
                    tc.tile_pool(name="tpsum", bufs=2, space="PSUM")
                )
                epool = actx.enter_context(tc.tile_pool(name="e", bufs=3))
                pspool = actx.enter_context(tc.tile_pool(name="psums", bufs=2))
                fpool = actx.enter_context(tc.tile_pool(name="fin", bufs=2))
                smpool = actx.enter_context(tc.tile_pool(name="small", bufs=2))

                for c in range(NSQ):
                    qsl = slice(c * SQC, (c + 1) * SQC)
                    o1_ps = opsum.tile([128, SQC], F32, tag="o1")
                    o2_ps = opsum.tile([128, SQC], F32, tag="o2")
                    p1 = pspool.tile([128, SQC], F32, tag="p1")
                    p2 = pspool.tile([128, SQC], F32, tag="p2")

                    for skt in range(NSK):
                        ksl = slice(skt * 128, (skt + 1) * 128)
                        for comp, (ktc, pacc, o_ps, etag) in enumerate(
                            [(kt1, p1, o1_ps, "e1"), (kt2, p2, o2_ps, "e2")]
                        ):
                            s_ps = spsum.tile([128, SQC], F32, tag=f"s{comp}")
                            nc.tensor.matmul(
                                s_ps[:],
                                ktc[:, ksl],
                                qt1[:, qsl] if comp == 0 else qt2[:, qsl],
                                start=True,
                                stop=True,
                            )
                            e_t = epool.tile([128, SQC], MM_DT, tag=etag)
                            nc.scalar.activation(
                                e_t[:],
                                s_ps[:],
                                mybir.ActivationFunctionType.Exp,
                                scale=float(INV_SQRT_D),
                            )
                            if skt == 0:
                                nc.vector.tensor_copy(pacc[:], e_t[:].bitcast(F32) if MM_F32R else e_t[:])
                            else:
                                nc.vector.tensor_add(pacc[:], pacc[:], e_t[:].bitcast(F32) if MM_F32R else e_t[:])
                            nc.tensor.matmul(
                                o_ps[:],
                                vv[:, ksl],
                                e_t[:],
                                start=(skt == 0),
                                stop=(skt == NSK - 1),
                            )

                    # ---- finalize this query chunk ----
                    o1_s = fpool.tile([128, SQC], F32, tag="o1s")
                    o2_s = fpool.tile([128, SQC], F32, tag="o2s")
                    nc.vector.tensor_copy(o1_s[:], o1_ps[:])
                    nc.vector.tensor_copy(o2_s[:], o2_ps[:])

                    inv1 = smpool.tile([128, 4], F32, tag="inv1")
                    inv2l = smpool.tile([128, 4], F32, tag="inv2l")
                    rs = smpool.tile([128, 2], F32, tag="rs")
                    for j in range(4):
                        jsl = slice(j * 128, (j + 1) * 128)
                        tr = tpsum.tile([128, 128], F32, tag="tr")
                        nc.tensor.transpose(tr[:], p1[:, jsl], ident[:])
                        nc.vector.reduce_sum(
                            rs[:, 0:1], tr[:], axis=mybir.AxisListType.X
                        )
                        nc.vector.reciprocal(inv1[:, j : j + 1], rs[:, 0:1])
                        tr2 = tpsum.tile([128, 128], F32, tag="tr")
                        nc.tensor.transpose(tr2[:], p2[:, jsl], ident[:])
                        nc.vector.reduce_sum(
                            rs[:, 1:2], tr2[:], axis=mybir.AxisListType.X
                        )
                        nc.vector.reciprocal(rs[:, 1:2], rs[:, 1:2])
                        nc.vector.tensor_mul(
                            inv2l[:, j : j + 1], rs[:, 1:2], lam_t[:, 0:1]
                        )

                    for j in range(4):
                        jsl = slice(j * 128, (j + 1) * 128)
                        tr1 = tpsum.tile([128, 128], F32, tag="tr")
                        nc.tensor.transpose(tr1[:], o1_s[:, jsl], ident[:])
                        tr2 = tpsum.tile([128, 128], F32, tag="tr")
                        nc.tensor.transpose(tr2[:], o2_s[:, jsl], ident[:])
                        tmp = fpool.tile([128, 128], F32, tag="tmp")
                        nc.vector.tensor_scalar_mul(
                            tmp[:], tr2[:], inv2l[:, j : j + 1]
                        )
                        o_t = fpool.tile([128, 128], F32, tag="ot")
                        nc.vector.scalar_tensor_tensor(
                            o_t[:],
                            tr1[:],
                            inv1[:, j : j + 1],
                            tmp[:],
                            op0=mybir.AluOpType.mult,
                            op1=mybir.AluOpType.subtract,
                        )
                        row = c * SQC + j * 128
                        nc.sync.dma_start(out=out[row : row + 128, :], in_=o_t[:])

    nc.compile()
    return nc


_NC = None


def _get_nc():
    global _NC
    if _NC is None:
        _NC = _build()
    return _NC


def kernel(X, lam, Wq, bq, Wk, bk, Wv, bv):
    X = np.asarray(X, dtype=np.float32)
    lam_f = float(np.asarray(lam))
    Wq = np.ascontiguousarray(np.asarray(Wq, np.float32))
    Wk = np.ascontiguousarray(np.asarray(Wk, np.float32))
    Wv = np.ascontiguousarray(np.asarray(Wv, np.float32))
    bq_c = np.asarray(bq, np.float32).reshape(2 * D, 1).copy()
    bk_c = np.asarray(bk, np.float32).reshape(2 * D, 1).copy()
    bv_c = np.asarray(bv, np.float32).reshape(D, 1).copy()
    lam_v = np.full((128, 1), lam_f, np.float32)

    nc = _get_nc()

    in_maps = []
    for core in range(8):
        b, h = divmod(core, 2)
        xb = X[b]
        if h == 0:
            xr = xb
        else:
            xr = np.concatenate([xb[NQ:], xb[:NQ]], axis=0)
        xt = np.ascontiguousarray(xr.T)
        in_maps.append(
            {
                "xt": xt,
                "wq": Wq,
                "wk": Wk,
                "wv": Wv,
                "bq": bq_c,
                "bk": bk_c,
                "bv": bv_c,
                "lamv": lam_v,
            }
        )

    global LAST_RESULT
    kwargs = {}
    if TRACE:
        import tempfile

        tdir = tempfile.mkdtemp(dir=TRACE_DIR) if TRACE_DIR else None
        kwargs = dict(trace=True, tmpdir=tdir)
    res = run_bass_kernel_spmd(nc, in_maps, list(range(8)), **kwargs)
    LAST_RESULT = res

    out = np.empty((B, S, D), np.float32)
    for core in range(8):
        b, h = divmod(core, 2)
        out[b, h * NQ : (h + 1) * NQ, :] = res.results[core]["o"]
    return out
